# revision 33
# baseline (speedup 1.0000x reference)
"""GQA attention (B=2,S=2048,E=2048,H=32,KVH=8,D=64, RoPE, non-causal) on 8 TRN2 cores.

Sharding: core = 4*b + g  (b = batch, g = head-group).  Each core owns one batch
and 8 q-heads / 2 kv-heads, computes a partial output projection; host sums the
4 group partials per batch.

v3 layout (all bf16 on-chip, f32 PSUM accum):
  xT[e,s] @ W*T -> QT/KT/VT feature-major via N=1024 streams
  RoPE on QT/KT (DVE elementwise, 1024-wide halves)
  V transposed to seq-major [k,64+1] via PE is_transpose matmuls
  Main loop: 16 pair-iterations (qc-major, fc-minor); each handles the head
  pair (2fc, 2fc+1) which shares one kv head and one 128-row q-feature tile.
    scores: per kc, TWO row-tiled matmuls (tile_position rows 0/64) run
      concurrently on the PE -> sp[128,1024] = [kpos, headA q | headB q]
    exp on ACT (one [128,1024] activation per kc)
    PV(prev pair) woven between score chunks; outproj + projection
      background work woven at fixed points so PE never idles
  normalize: DVE reciprocal_approx_fast + gpsimd partition_broadcast + DVE mul
  out^T = attnT . WoT, staged bf16, host sums partials in f32.
"""

import numpy as np
import ml_dtypes

import concourse.bass as bass
import concourse.tile as tile
from concourse import bacc, mybir
from concourse.bass_utils import run_bass_kernel_spmd

BF16 = ml_dtypes.bfloat16
F32 = mybir.dt.float32
BF = mybir.dt.bfloat16

B, S, E = 2, 2048, 2048
H, KVH, D = 32, 8, 64
N_CORES = 8
FH = 512          # features per core (8 heads * 64)
EC = 16           # e-chunks (128)

_CACHE = {}


def _build():
    nc = bacc.Bacc("TRN2", target_bir_lowering=False, debug=False,
                   num_devices=N_CORES)
    xt_d = nc.dram_tensor("xt", [E, S], BF, kind="ExternalInput")
    # weights pre-packed host-side as [128, chunks*cols] so each load is one
    # DMA with 4KB contiguous rows (small-descriptor DMAs were setup-bound)
    wqt_d = nc.dram_tensor("wqt", [128, 4 * E], BF, kind="ExternalInput")
    wkt_d = nc.dram_tensor("wkt", [128, E], BF, kind="ExternalInput")
    wvt_d = nc.dram_tensor("wvt", [128, E], BF, kind="ExternalInput")
    wot_d = nc.dram_tensor("wot", [FH, E], BF, kind="ExternalInput")
    cos_d = nc.dram_tensor("cost", [128, S], BF, kind="ExternalInput")
    nsin_d = nc.dram_tensor("nsint", [128, S], BF, kind="ExternalInput")
    eye_d = nc.dram_tensor("eye", [128, 128], BF, kind="ExternalInput")
    out_d = nc.dram_tensor("out", [S, E], BF, kind="ExternalOutput")
    dbg_d = nc.dram_tensor("dbg", [128, 512], F32, kind="ExternalOutput")

    from contextlib import ExitStack
    with ExitStack() as ctx:
        tc = ctx.enter_context(tile.TileContext(nc))
        pool = lambda *a, **k: ctx.enter_context(tc.tile_pool(*a, **k))
        xt_p = pool(name="xt", bufs=16)
        wq_p = pool(name="wq", bufs=2)       # lazy per-fc q-weight ring
        wk_p = pool(name="wk", bufs=1)
        wv_p = pool(name="wv", bufs=1)
        wot_p = pool(name="wot", bufs=4)
        cs_p = pool(name="cs", bufs=2)
        eye_p = pool(name="eye", bufs=1)
        raw_p = pool(name="raw", bufs=2)     # kraw / qraw / vf rotate here
        t2_p = pool(name="t2", bufs=2)       # rope temps, [128,1024]
        qkt_p = pool(name="qkt", bufs=6)     # qt x4 + ktd x2
        vt_p = pool(name="vt", bufs=32)
        pt_p = pool(name="pt", bufs=16)
        rc_p = pool(name="rc", bufs=2)
        rds_p = pool(name="rds", bufs=2)
        rdb_p = pool(name="rdb", bufs=1, space="PSUM")
        nt_p = pool(name="nt", bufs=2)
        at_p = pool(name="at", bufs=3)
        ost_p = pool(name="ost", bufs=2)
        sp_p = pool(name="sp", bufs=2, space="PSUM")   # [128,1024] scores/proj/op
        pv_p = pool(name="pv", bufs=3, space="PSUM")   # [128,512] pv pairs (+tp)

        # ---- initial DMA loads ----
        # sync queue carries the critical path (K weights, rope tables, x,
        # Q0/Q1 weights); the idle ACT queue issues the rest (V, Wo, eye)
        wk_t = wk_p.tile([128, E], BF, tag="wk", name="wk_t")
        nc.sync.dma_start(wk_t[:], wkt_d[:, :])
        wkt = [wk_t[:, 128 * i:128 * (i + 1)] for i in range(EC)]
        cos_t = cs_p.tile([128, S], BF, tag="cs", name="cos_t")
        nc.sync.dma_start(cos_t[:], cos_d[:, :])
        nsin_t = cs_p.tile([128, S], BF, tag="cs", name="nsin_t")
        nc.sync.dma_start(nsin_t[:], nsin_d[:, :])
        xt = []
        for i in range(EC):
            t = xt_p.tile([128, S], BF, tag="xt", name=f"xt{i}")
            nc.sync.dma_start(t[:], xt_d[128 * i:128 * (i + 1), :])
            xt.append(t)

        def load_wq(fc):
            t = wq_p.tile([128, E], BF, tag="wq", name=f"wq{fc}")
            nc.sync.dma_start(t[:], wqt_d[:, E * fc:E * (fc + 1)])
            return [t[:, 128 * i:128 * (i + 1)] for i in range(EC)]

        wq_tiles = {0: load_wq(0), 1: load_wq(1)}
        wv_t = wv_p.tile([128, E], BF, tag="wv", name="wv_t")
        nc.scalar.dma_start(wv_t[:], wvt_d[:, :])
        wvt = [wv_t[:, 128 * i:128 * (i + 1)] for i in range(EC)]
        wot = []
        for i in range(4):
            t = wot_p.tile([128, E], BF, tag="wot", name=f"wot{i}")
            nc.scalar.dma_start(t[:], wot_d[128 * i:128 * (i + 1), :])
            wot.append(t)
        eye_t = eye_p.tile([128, 128], BF, tag="eye", name="eye_t")
        nc.scalar.dma_start(eye_t[:], eye_d[:, :])
        # ones row on partition 64: stationary operand for the reciprocal
        # partition-broadcast matmul (k=1 at array row 64 -> psum rows 0:64)
        ones_t = eye_p.tile([65, 64], BF, tag="ones", name="ones_t")
        nc.vector.memset(ones_t[64:65, :], 1.0)

        def rope(src, dst):
            # dst = src*cos + shift32(src)*nsin, per 64-row head block,
            # processed in two 1024-wide halves (smaller SBUF temps).
            # partition shift must go through DMA (engines are lane-locked)
            for h2 in range(2):
                sl = slice(1024 * h2, 1024 * (h2 + 1))
                qs = t2_p.tile([128, 1024], BF, tag="t2", name="qs")
                for blk in (0, 64):
                    nc.sync.dma_start(qs[blk:blk + 32, :],
                                      src[blk + 32:blk + 64, sl])
                    nc.sync.dma_start(qs[blk + 32:blk + 64, :],
                                      src[blk:blk + 32, sl])
                t2 = t2_p.tile([128, 1024], BF, tag="t2", name="t2")
                nc.vector.tensor_mul(t2[:], qs[:], nsin_t[:, sl])
                nc.vector.tensor_mul(dst[:, sl], src[:, sl], cos_t[:, sl])
                nc.vector.tensor_add(dst[:, sl], dst[:, sl], t2[:])

        def proj_sh(wtiles, dst, sh):
            # one 1024-wide half of a feature-major projection
            ps = sp_p.tile([128, 1024], F32, tag="sp", name="ps")
            for j in range(2):
                ssl_j = slice(1024 * sh + 512 * j, 1024 * sh + 512 * (j + 1))
                for ec in range(EC):
                    nc.tensor.matmul(ps[:, 512 * j:512 * (j + 1)],
                                     wtiles[ec][:, :],
                                     xt[ec][:, ssl_j],
                                     start=(ec == 0), stop=(ec == EC - 1))
            nc.vector.tensor_copy(dst[:, 1024 * sh:1024 * (sh + 1)], ps[:])

        # ---- K projection + rope + dup (setup) ----
        kraw = raw_p.tile([128, S], BF, tag="raw", name="kraw")
        proj_sh(wkt, kraw, 0)
        proj_sh(wkt, kraw, 1)
        rope(kraw, kraw)
        # duplicate each kv head across both partition halves so the
        # row-tiled scores pair can read its kv head at rows 0:64 and 64:128
        ktd = [qkt_p.tile([128, S], BF, tag="qkt", name=f"ktd{i}")
               for i in range(2)]
        for kv in range(2):
            src = kraw[64 * kv:64 * (kv + 1), :]
            nc.sync.dma_start(ktd[kv][0:64, :], src)
            nc.sync.dma_start(ktd[kv][64:128, :], src)

        # ---- Q0 projection + rope (setup) ----
        qt = [qkt_p.tile([128, S], BF, tag="qkt", name=f"qt{fc}")
              for fc in range(4)]
        qraw0 = raw_p.tile([128, S], BF, tag="raw", name="qraw0")
        proj_sh(wq_tiles[0], qraw0, 0)
        proj_sh(wq_tiles[0], qraw0, 1)
        rope(qraw0, qt[0])

        # ---- background work items woven into the attention loop ----
        vt = {}     # (kc, kv_local) -> [128, 65]  (col 64 = ones)
        state = {}

        def bg_v_sh(sh):
            if sh == 0:
                state["vf"] = raw_p.tile([128, S], BF, tag="raw", name="vf")
            proj_sh(wvt, state["vf"], sh)

        def bg_build_vt():
            vf = state["vf"]
            for kc in range(16):
                tp = pv_p.tile([128, 128], BF, tag="pv", name="tp")
                nc.tensor.matmul(tp[:], vf[:, 128 * kc:128 * (kc + 1)],
                                 eye_t[:], is_transpose=True)
                for kv in range(2):
                    v = vt_p.tile([128, 65], BF, tag="vt", name=f"v{kc}_{kv}")
                    nc.vector.tensor_copy(v[:, 0:64],
                                          tp[:, 64 * kv:64 * (kv + 1)])
                    nc.gpsimd.memset(v[:, 64:65], 1.0)
                    vt[(kc, kv)] = v

        def bg_q_sh(fc, sh):
            if sh == 0:
                state[f"qraw{fc}"] = raw_p.tile([128, S], BF, tag="raw",
                                                name=f"qraw{fc}")
            proj_sh(wq_tiles[fc], state[f"qraw{fc}"], sh)

        def bg_q_rope(fc):
            rope(state[f"qraw{fc}"], qt[fc])

        def bg_load_wq(fc):
            wq_tiles[fc] = load_wq(fc)

        # per-pair-iteration background queues (drained at 4 weave points)
        bg_by_it = {
            0: [lambda: bg_v_sh(0), lambda: bg_v_sh(1), bg_build_vt,
                lambda: bg_load_wq(2),
                lambda: bg_q_sh(1, 0), lambda: bg_q_sh(1, 1),
                lambda: bg_q_rope(1)],
            1: [lambda: bg_load_wq(3),
                lambda: bg_q_sh(2, 0), lambda: bg_q_sh(2, 1),
                lambda: bg_q_rope(2)],
            2: [lambda: bg_q_sh(3, 0), lambda: bg_q_sh(3, 1),
                lambda: bg_q_rope(3)],
        }

        # ---- attention: 16 pair-iterations ----
        EXP = mybir.ActivationFunctionType.Exp
        attnt = {}     # qc -> [4 tiles [128,512] bf16]
        oqueue = []    # pending outproj groups: (qc, sti, ecb)

        ocount = [0]

        def weave_outproj():
            if not oqueue:
                return
            oqc, sti, ecb = oqueue.pop(0)
            st = 4 * oqc + sti
            op = sp_p.tile([128, 512], F32, tag="sp", name="op")
            for fc2 in range(4):
                nc.tensor.matmul(
                    op[:],
                    attnt[oqc][fc2][:, 128 * sti:128 * (sti + 1)],
                    wot[fc2][:, 512 * ecb:512 * (ecb + 1)],
                    start=(fc2 == 0), stop=(fc2 == 3),
                    skip_group_check=True)
            so = ost_p.tile([128, 512], BF, tag="ost", name="so")
            # psum->sbuf copies alternate ACT/DVE (gpsimd can't read PSUM)
            ocount[0] += 1
            if ocount[0] % 2:
                nc.scalar.copy(so[:], op[:])
            else:
                nc.vector.tensor_copy(so[:], op[:])
            nc.gpsimd.dma_start(
                out_d[128 * st:128 * (st + 1),
                      512 * ecb:512 * (ecb + 1)], so[:])

        def weave_pv(prev, kc):
            ppts, ppvA, ppvB, pkv = prev
            nc.tensor.matmul(
                ppvA[0:65, :], vt[(kc, pkv)][:, 0:65],
                ppts[kc][:, 0:512],
                start=(kc == 0), stop=(kc == 15),
                skip_group_check=True)
            nc.tensor.matmul(
                ppvB[0:65, :], vt[(kc, pkv)][:, 0:65],
                ppts[kc][:, 512:1024],
                start=(kc == 0), stop=(kc == 15),
                skip_group_check=True)

        def emit_recip(ppv):
            # recip on partition 64 (DVE lanes are partition-locked), cast
            # to bf16 for a cheap PE broadcast later
            rc = rc_p.tile([65, 512], F32, tag="rc", name="rc")
            nc.vector.reciprocal(rc[64:65, :], ppv[64:65, :])
            rcb = rc_p.tile([65, 512], BF, tag="rcb", name="rcb")
            nc.vector.tensor_copy(rcb[64:65, :], rc[64:65, :])
            return rcb

        def emit_bcast(rcb):
            # broadcast partition 64 -> psum rows 0:64 via a k=1 matmul with
            # the ones row as stationary operand (array row 64), then stage
            # to SBUF (DVE may read only one PSUM operand per instruction)
            rp = rdb_p.tile([64, 512], F32, tag="rdb", name="rp")
            nc.tensor.matmul(rp[:], ones_t[64:65, :], rcb[64:65, :],
                             start=True, stop=True, skip_group_check=True)
            rdb = rds_p.tile([64, 512], BF, tag="rds", name="rdb")
            nc.vector.tensor_copy(rdb[:], rp[:])
            return rdb

        def emit_mul_A(i, ppvA, rdbA):
            qc, fc = divmod(i, 4)
            nc.vector.tensor_mul(attnt[qc][fc][0:64, :], ppvA[0:64, :],
                                 rdbA[:])

        def emit_mul_B(i, ppvB, rdbB):
            qc, fc = divmod(i, 4)
            nt = nt_p.tile([64, 512], BF, tag="nt", name="nt")
            nc.vector.tensor_mul(nt[:], ppvB[0:64, :], rdbB[:])
            nc.sync.dma_start(attnt[qc][fc][64:128, :], nt[:])
            if fc == 3:
                oqueue.extend((qc, sti, ecb)
                              for sti in range(4) for ecb in range(4))

        def emit_pair_scores(i, prev, bgq):
            qc, fc = divmod(i, 4)
            kv = fc // 2
            qsl = slice(512 * qc, 512 * (qc + 1))
            if fc == 0:
                attnt[qc] = [at_p.tile([128, 512], BF, tag=f"at{j}",
                                       name=f"attnt{j}_{qc}")
                             for j in range(4)]
            pts = []
            for kc in range(16):
                # front-load the PV weave (4 pairs per step over the first 4
                # steps) so the normalize starts early and finishes mid-it
                if prev is not None and kc < 4:
                    for j in range(4):
                        weave_pv(prev, 4 * kc + j)
                if prev is not None:
                    # staggered normalize for pair i-1 (pv complete after
                    # step 3): recips+casts early, broadcast+mul when ready
                    if kc == 4:
                        state["rcbA"] = emit_recip(prev[1])
                        state["rcbB"] = emit_recip(prev[2])
                    elif kc == 10:
                        rdbA = emit_bcast(state["rcbA"])
                        emit_mul_A(i - 1, prev[1], rdbA)
                    elif kc == 12:
                        rdbB = emit_bcast(state["rcbB"])
                        emit_mul_B(i - 1, prev[2], rdbB)
                if kc in (9, 11, 13, 15):
                    if oqueue:
                        weave_outproj()
                    if bgq:
                        n = 2 if len(bgq) > 4 else 1
                        for _ in range(n):
                            if bgq:
                                bgq.pop(0)()
                # row-tiled score pair: head A rows 0:64, head B rows 64:128
                sp = sp_p.tile([128, 1024], F32, tag="sp", name="sp")
                nc.tensor.matmul(sp[:, 0:512],
                                 ktd[kv][0:64, 128 * kc:128 * (kc + 1)],
                                 qt[fc][0:64, qsl], start=True, stop=True)
                nc.tensor.matmul(sp[:, 512:1024],
                                 ktd[kv][64:128, 128 * kc:128 * (kc + 1)],
                                 qt[fc][64:128, qsl], start=True, stop=True)
                pt = pt_p.tile([128, 1024], BF, tag="pt", name="pt")
                nc.scalar.activation(pt[:], sp[:], EXP, 0.0, 0.125)
                pts.append(pt)
            while bgq:
                bgq.pop(0)()
            return pts

        prev = None    # (pts, pvA, pvB, kv) of pair-it i-1
        for i in range(16):
            qc, fc = divmod(i, 4)
            pts = emit_pair_scores(i, prev, bg_by_it.get(i))
            pvA = pv_p.tile([128, 512], F32, tag="pv", name=f"pvA{i}")
            pvB = pv_p.tile([128, 512], F32, tag="pv", name=f"pvB{i}")
            prev = (pts, pvA, pvB, fc // 2)
        # flush: PV(15), norm, muls, remaining outproj
        for kc in range(16):
            if kc % 3 == 0 and oqueue:
                weave_outproj()
            weave_pv(prev, kc)
        rcbA = emit_recip(prev[1])
        rcbB = emit_recip(prev[2])
        rdbA = emit_bcast(rcbA)
        emit_mul_A(15, prev[1], rdbA)
        rdbB = emit_bcast(rcbB)
        emit_mul_B(15, prev[2], rdbB)
        while oqueue:
            weave_outproj()

    nc.compile()
    return nc


def _tables():
    inv = 1.0 / (10000.0 ** (np.arange(0, 64, 2, dtype=np.float64) / 64))
    t = np.arange(S, dtype=np.float64)
    emb = np.concatenate([np.outer(t, inv)] * 2, -1)          # [S,64]
    cos_t = np.cos(emb).T.astype(np.float32)                  # [64,S]
    sin_t = np.sin(emb).T.astype(np.float32)
    ssin = np.concatenate([-sin_t[:32], sin_t[32:]], 0)
    cos_tile = np.ascontiguousarray(np.vstack([cos_t, cos_t])).astype(BF16)
    nsin_tile = np.ascontiguousarray(np.vstack([ssin, ssin])).astype(BF16)
    return cos_tile, nsin_tile


def kernel(x, Wq, Wk, Wv, Wo):
    x = np.asarray(x, np.float32)
    Wq, Wk, Wv, Wo = (np.asarray(w, np.float32) for w in (Wq, Wk, Wv, Wo))
    if "nc" not in _CACHE:
        _CACHE["nc"] = _build()
    nc = _CACHE["nc"]
    cos_tile, nsin_tile = _tables()
    eye = np.eye(128, dtype=BF16)
    xts = [np.ascontiguousarray(x[b].T).astype(BF16) for b in range(B)]

    def pack_ecols(wt):
        # [E, C] -> [128, EC*C]: chunk ec, col c at [:, C*ec + c]; each SBUF
        # row is then one contiguous 4KB-ish DMA row
        ec, c = wt.shape[0] // 128, wt.shape[1]
        return np.ascontiguousarray(
            wt.reshape(ec, 128, c).transpose(1, 0, 2).reshape(128, ec * c))

    in_maps = []
    for core in range(N_CORES):
        b, g = divmod(core, 4)
        fsl = slice(FH * g, FH * (g + 1))
        dsl = slice(128 * g, 128 * (g + 1))
        wqt = Wq[fsl].T  # [E, 512]
        # per-fc packed blocks side by side: [:, E*fc + 128*ec + c]
        wq_pack = np.concatenate(
            [pack_ecols(wqt[:, 128 * fc:128 * (fc + 1)]) for fc in range(4)],
            axis=1)
        in_maps.append({
            "xt": xts[b],
            "wqt": wq_pack.astype(BF16),
            "wkt": pack_ecols(Wk[dsl].T).astype(BF16),
            "wvt": pack_ecols(Wv[dsl].T).astype(BF16),
            "wot": np.ascontiguousarray(Wo[:, fsl].T).astype(BF16),
            "cost": cos_tile,
            "nsint": nsin_tile,
            "eye": eye,
        })
    res = run_bass_kernel_spmd(nc, in_maps, core_ids=list(range(N_CORES)),
                               **_CACHE.get("run_kwargs", {}))
    _CACHE["last_result"] = res
    _CACHE["dbg"] = [r.get("dbg") for r in res.results]
    out = np.empty((B, S, E), np.float32)
    for b in range(B):
        out[b] = sum(res.results[4 * b + g]["out"].astype(np.float32)
                     for g in range(4))
    return out


# revision 34
# speedup vs baseline: 1.0694x; 1.0694x over previous
"""GQA attention (B=2,S=2048,E=2048,H=32,KVH=8,D=64, RoPE, non-causal) on 8 TRN2 cores.

Sharding: core = 4*b + g  (b = batch, g = head-group).  Each core owns one batch
and 8 q-heads / 2 kv-heads, computes a partial output projection; host sums the
4 group partials per batch.

v3 layout (all bf16 on-chip, f32 PSUM accum):
  xT[e,s] @ W*T -> QT/KT/VT feature-major via N=1024 streams
  RoPE on QT/KT (DVE elementwise, 1024-wide halves)
  V transposed to seq-major [k,64+1] via PE is_transpose matmuls
  Main loop: 16 pair-iterations (qc-major, fc-minor); each handles the head
  pair (2fc, 2fc+1) which shares one kv head and one 128-row q-feature tile.
    scores: per kc, TWO row-tiled matmuls (tile_position rows 0/64) run
      concurrently on the PE -> sp[128,1024] = [kpos, headA q | headB q]
    exp on ACT (one [128,1024] activation per kc)
    PV(prev pair) woven between score chunks; outproj + projection
      background work woven at fixed points so PE never idles
  normalize: DVE reciprocal_approx_fast + gpsimd partition_broadcast + DVE mul
  out^T = attnT . WoT, staged bf16, host sums partials in f32.
"""

import numpy as np
import ml_dtypes

import concourse.bass as bass
import concourse.tile as tile
from concourse import bacc, mybir
from concourse.bass_utils import run_bass_kernel_spmd

BF16 = ml_dtypes.bfloat16
F32 = mybir.dt.float32
BF = mybir.dt.bfloat16

B, S, E = 2, 2048, 2048
H, KVH, D = 32, 8, 64
N_CORES = 8
FH = 512          # features per core (8 heads * 64)
EC = 16           # e-chunks (128)

_CACHE = {}


def _build():
    nc = bacc.Bacc("TRN2", target_bir_lowering=False, debug=False,
                   num_devices=N_CORES)
    xt_d = nc.dram_tensor("xt", [E, S], BF, kind="ExternalInput")
    # weights pre-packed host-side as [128, chunks*cols] so each load is one
    # DMA with 4KB contiguous rows (small-descriptor DMAs were setup-bound)
    wqt_d = nc.dram_tensor("wqt", [128, 4 * E], BF, kind="ExternalInput")
    wkt_d = nc.dram_tensor("wkt", [128, E], BF, kind="ExternalInput")
    wvt_d = nc.dram_tensor("wvt", [128, E], BF, kind="ExternalInput")
    wot_d = nc.dram_tensor("wot", [FH, E], BF, kind="ExternalInput")
    cos_d = nc.dram_tensor("cost", [128, S], BF, kind="ExternalInput")
    nsin_d = nc.dram_tensor("nsint", [128, S], BF, kind="ExternalInput")
    eye_d = nc.dram_tensor("eye", [128, 128], BF, kind="ExternalInput")
    out_d = nc.dram_tensor("out", [S, E], BF, kind="ExternalOutput")
    dbg_d = nc.dram_tensor("dbg", [128, 512], F32, kind="ExternalOutput")

    from contextlib import ExitStack
    with ExitStack() as ctx:
        tc = ctx.enter_context(tile.TileContext(nc))
        pool = lambda *a, **k: ctx.enter_context(tc.tile_pool(*a, **k))
        xt_p = pool(name="xt", bufs=16)
        wq_p = pool(name="wq", bufs=2)       # lazy per-fc q-weight ring
        wk_p = pool(name="wk", bufs=1)
        wv_p = pool(name="wv", bufs=1)
        wot_p = pool(name="wot", bufs=4)
        cs_p = pool(name="cs", bufs=2)
        eye_p = pool(name="eye", bufs=1)
        raw_p = pool(name="raw", bufs=2)     # kraw / qraw / vf rotate here
        t2_p = pool(name="t2", bufs=2)       # rope temps, [128,1024]
        qkt_p = pool(name="qkt", bufs=6)     # qt x4 + ktd x2
        vt_p = pool(name="vt", bufs=32)
        pt_p = pool(name="pt", bufs=16)
        rc_p = pool(name="rc", bufs=2)
        rds_p = pool(name="rds", bufs=2)
        nt_p = pool(name="nt", bufs=2)
        at_p = pool(name="at", bufs=3)
        ost_p = pool(name="ost", bufs=2)
        sp_p = pool(name="sp", bufs=2, space="PSUM")   # [128,1024] scores/proj/op
        pv_p = pool(name="pv", bufs=4, space="PSUM")   # [128,512] pv pairs (+tp)

        # ---- initial DMA loads ----
        # sync queue carries the critical path (K weights, rope tables, x,
        # Q0/Q1 weights); the idle ACT queue issues the rest (V, Wo, eye)
        wk_t = wk_p.tile([128, E], BF, tag="wk", name="wk_t")
        nc.sync.dma_start(wk_t[:], wkt_d[:, :])
        wkt = [wk_t[:, 128 * i:128 * (i + 1)] for i in range(EC)]
        cos_t = cs_p.tile([128, S], BF, tag="cs", name="cos_t")
        nc.sync.dma_start(cos_t[:], cos_d[:, :])
        nsin_t = cs_p.tile([128, S], BF, tag="cs", name="nsin_t")
        nc.sync.dma_start(nsin_t[:], nsin_d[:, :])
        xt = []
        for i in range(EC):
            t = xt_p.tile([128, S], BF, tag="xt", name=f"xt{i}")
            nc.sync.dma_start(t[:], xt_d[128 * i:128 * (i + 1), :])
            xt.append(t)

        def load_wq(fc):
            t = wq_p.tile([128, E], BF, tag="wq", name=f"wq{fc}")
            nc.sync.dma_start(t[:], wqt_d[:, E * fc:E * (fc + 1)])
            return [t[:, 128 * i:128 * (i + 1)] for i in range(EC)]

        wq_tiles = {0: load_wq(0), 1: load_wq(1)}
        wv_t = wv_p.tile([128, E], BF, tag="wv", name="wv_t")
        nc.scalar.dma_start(wv_t[:], wvt_d[:, :])
        wvt = [wv_t[:, 128 * i:128 * (i + 1)] for i in range(EC)]
        wot = []
        for i in range(4):
            t = wot_p.tile([128, E], BF, tag="wot", name=f"wot{i}")
            nc.scalar.dma_start(t[:], wot_d[128 * i:128 * (i + 1), :])
            wot.append(t)
        eye_t = eye_p.tile([128, 128], BF, tag="eye", name="eye_t")
        nc.scalar.dma_start(eye_t[:], eye_d[:, :])
        # ones row on partition 64: stationary operand for the reciprocal
        # partition-broadcast matmul (k=1 at array row 64 -> psum rows 0:64)
        ones_t = eye_p.tile([65, 64], BF, tag="ones", name="ones_t")
        nc.vector.memset(ones_t[64:65, :], 1.0)

        def rope(src, dst):
            # dst = src*cos + shift32(src)*nsin, per 64-row head block,
            # processed in two 1024-wide halves (smaller SBUF temps).
            # partition shift must go through DMA (engines are lane-locked)
            for h2 in range(2):
                sl = slice(1024 * h2, 1024 * (h2 + 1))
                qs = t2_p.tile([128, 1024], BF, tag="t2", name="qs")
                for blk in (0, 64):
                    nc.sync.dma_start(qs[blk:blk + 32, :],
                                      src[blk + 32:blk + 64, sl])
                    nc.sync.dma_start(qs[blk + 32:blk + 64, :],
                                      src[blk:blk + 32, sl])
                t2 = t2_p.tile([128, 1024], BF, tag="t2", name="t2")
                nc.vector.tensor_mul(t2[:], qs[:], nsin_t[:, sl])
                nc.vector.tensor_mul(dst[:, sl], src[:, sl], cos_t[:, sl])
                nc.vector.tensor_add(dst[:, sl], dst[:, sl], t2[:])

        def proj_sh(wtiles, dst, sh):
            # one 1024-wide half of a feature-major projection
            ps = sp_p.tile([128, 1024], F32, tag="sp", name="ps")
            for j in range(2):
                ssl_j = slice(1024 * sh + 512 * j, 1024 * sh + 512 * (j + 1))
                for ec in range(EC):
                    nc.tensor.matmul(ps[:, 512 * j:512 * (j + 1)],
                                     wtiles[ec][:, :],
                                     xt[ec][:, ssl_j],
                                     start=(ec == 0), stop=(ec == EC - 1))
            nc.vector.tensor_copy(dst[:, 1024 * sh:1024 * (sh + 1)], ps[:])

        # ---- K projection + rope + dup (setup) ----
        kraw = raw_p.tile([128, S], BF, tag="raw", name="kraw")
        proj_sh(wkt, kraw, 0)
        proj_sh(wkt, kraw, 1)
        rope(kraw, kraw)
        # duplicate each kv head across both partition halves so the
        # row-tiled scores pair can read its kv head at rows 0:64 and 64:128
        ktd = [qkt_p.tile([128, S], BF, tag="qkt", name=f"ktd{i}")
               for i in range(2)]
        for kv in range(2):
            src = kraw[64 * kv:64 * (kv + 1), :]
            nc.sync.dma_start(ktd[kv][0:64, :], src)
            nc.sync.dma_start(ktd[kv][64:128, :], src)

        # ---- Q0 projection + rope (setup) ----
        qt = [qkt_p.tile([128, S], BF, tag="qkt", name=f"qt{fc}")
              for fc in range(4)]
        qraw0 = raw_p.tile([128, S], BF, tag="raw", name="qraw0")
        proj_sh(wq_tiles[0], qraw0, 0)
        proj_sh(wq_tiles[0], qraw0, 1)
        rope(qraw0, qt[0])

        # ---- background work items woven into the attention loop ----
        vt = {}     # (kc, kv_local) -> [128, 65]  (col 64 = ones)
        state = {}

        def bg_v_sh(sh):
            if sh == 0:
                state["vf"] = raw_p.tile([128, S], BF, tag="raw", name="vf")
            proj_sh(wvt, state["vf"], sh)

        def bg_build_vt():
            vf = state["vf"]
            for kc in range(16):
                tp = pv_p.tile([128, 128], BF, tag="pv", name="tp")
                nc.tensor.matmul(tp[:], vf[:, 128 * kc:128 * (kc + 1)],
                                 eye_t[:], is_transpose=True)
                for kv in range(2):
                    v = vt_p.tile([128, 65], BF, tag="vt", name=f"v{kc}_{kv}")
                    nc.vector.tensor_copy(v[:, 0:64],
                                          tp[:, 64 * kv:64 * (kv + 1)])
                    nc.gpsimd.memset(v[:, 64:65], 1.0)
                    vt[(kc, kv)] = v

        def bg_q_sh(fc, sh):
            if sh == 0:
                state[f"qraw{fc}"] = raw_p.tile([128, S], BF, tag="raw",
                                                name=f"qraw{fc}")
            proj_sh(wq_tiles[fc], state[f"qraw{fc}"], sh)

        def bg_q_rope(fc):
            rope(state[f"qraw{fc}"], qt[fc])

        def bg_load_wq(fc):
            wq_tiles[fc] = load_wq(fc)

        # per-pair-iteration background queues (drained at 4 weave points)
        bg_by_it = {
            0: [lambda: bg_v_sh(0), lambda: bg_v_sh(1), bg_build_vt,
                lambda: bg_load_wq(2),
                lambda: bg_q_sh(1, 0), lambda: bg_q_sh(1, 1),
                lambda: bg_q_rope(1)],
            1: [lambda: bg_load_wq(3),
                lambda: bg_q_sh(2, 0), lambda: bg_q_sh(2, 1),
                lambda: bg_q_rope(2)],
            2: [lambda: bg_q_sh(3, 0), lambda: bg_q_sh(3, 1),
                lambda: bg_q_rope(3)],
        }

        # ---- attention: 16 pair-iterations ----
        EXP = mybir.ActivationFunctionType.Exp
        attnt = {}     # qc -> [4 tiles [128,512] bf16]
        oqueue = []    # pending outproj groups: (qc, sti, ecb)

        ocount = [0]

        def weave_outproj():
            if not oqueue:
                return
            oqc, sti, ecb = oqueue.pop(0)
            st = 4 * oqc + sti
            op = sp_p.tile([128, 512], F32, tag="sp", name="op")
            for fc2 in range(4):
                nc.tensor.matmul(
                    op[:],
                    attnt[oqc][fc2][:, 128 * sti:128 * (sti + 1)],
                    wot[fc2][:, 512 * ecb:512 * (ecb + 1)],
                    start=(fc2 == 0), stop=(fc2 == 3),
                    skip_group_check=True)
            so = ost_p.tile([128, 512], BF, tag="ost", name="so")
            nc.vector.tensor_copy(so[:], op[:])
            nc.gpsimd.dma_start(
                out_d[128 * st:128 * (st + 1),
                      512 * ecb:512 * (ecb + 1)], so[:])

        def weave_pv(prev, kc):
            ppts, ppvA, ppvB, pkv = prev
            nc.tensor.matmul(
                ppvA[0:65, :], vt[(kc, pkv)][:, 0:65],
                ppts[kc][:, 0:512],
                start=(kc == 0), stop=(kc == 15),
                skip_group_check=True)
            nc.tensor.matmul(
                ppvB[0:65, :], vt[(kc, pkv)][:, 0:65],
                ppts[kc][:, 512:1024],
                start=(kc == 0), stop=(kc == 15),
                skip_group_check=True)

        def emit_recip(ppv):
            # recip on partition 64 (DVE lanes are partition-locked), cast
            # to bf16 for a cheap PE broadcast later
            rc = rc_p.tile([65, 512], F32, tag="rc", name="rc")
            nc.vector.reciprocal(rc[64:65, :], ppv[64:65, :])
            rcb = rc_p.tile([65, 512], BF, tag="rcb", name="rcb")
            nc.vector.tensor_copy(rcb[64:65, :], rc[64:65, :])
            return rcb

        def emit_bcast(rcb):
            # broadcast partition 64 -> psum rows 0:64 via a k=1 matmul with
            # the ones row as stationary operand (array row 64), then stage
            # to SBUF (DVE may read only one PSUM operand per instruction)
            rp = sp_p.tile([64, 512], F32, tag="sp", name="rp")
            nc.tensor.matmul(rp[:], ones_t[64:65, :], rcb[64:65, :],
                             start=True, stop=True, skip_group_check=True)
            rdb = rds_p.tile([64, 512], BF, tag="rds", name="rdb")
            nc.vector.tensor_copy(rdb[:], rp[:])
            return rdb

        def emit_mul_A(i, ppvA, rdbA):
            qc, fc = divmod(i, 4)
            nc.vector.tensor_mul(attnt[qc][fc][0:64, :], ppvA[0:64, :],
                                 rdbA[:])

        def emit_mul_B(i, ppvB, rdbB):
            qc, fc = divmod(i, 4)
            nt = nt_p.tile([64, 512], BF, tag="nt", name="nt")
            nc.vector.tensor_mul(nt[:], ppvB[0:64, :], rdbB[:])
            nc.sync.dma_start(attnt[qc][fc][64:128, :], nt[:])
            if fc == 3:
                oqueue.extend((qc, sti, ecb)
                              for sti in range(4) for ecb in range(4))

        def emit_pair_scores(i, prev, bgq):
            qc, fc = divmod(i, 4)
            kv = fc // 2
            qsl = slice(512 * qc, 512 * (qc + 1))
            if fc == 0:
                attnt[qc] = [at_p.tile([128, 512], BF, tag=f"at{j}",
                                       name=f"attnt{j}_{qc}")
                             for j in range(4)]
            pts = []
            for kc in range(16):
                # PV weave for pair i-1: 2 pairs per step over steps 0-7
                # (pv ring is 4 deep, so the normalize chain has a full
                # iteration of slack and never stalls the PE)
                if prev is not None and kc < 8:
                    weave_pv(prev, 2 * kc)
                    weave_pv(prev, 2 * kc + 1)
                if prev is not None:
                    if kc == 8:
                        state["rcbA"] = emit_recip(prev[1])
                    elif kc == 10:
                        state["rcbB"] = emit_recip(prev[2])
                    elif kc == 12:
                        rdbA = emit_bcast(state["rcbA"])
                        emit_mul_A(i - 1, prev[1], rdbA)
                    elif kc == 15:
                        rdbB = emit_bcast(state["rcbB"])
                        emit_mul_B(i - 1, prev[2], rdbB)
                if kc in (2, 4, 6, 13):
                    if oqueue:
                        weave_outproj()
                    if bgq:
                        n = 2 if len(bgq) > 4 else 1
                        for _ in range(n):
                            if bgq:
                                bgq.pop(0)()
                # row-tiled score pair: head A rows 0:64, head B rows 64:128
                sp = sp_p.tile([128, 1024], F32, tag="sp", name="sp")
                nc.tensor.matmul(sp[:, 0:512],
                                 ktd[kv][0:64, 128 * kc:128 * (kc + 1)],
                                 qt[fc][0:64, qsl], start=True, stop=True)
                nc.tensor.matmul(sp[:, 512:1024],
                                 ktd[kv][64:128, 128 * kc:128 * (kc + 1)],
                                 qt[fc][64:128, qsl], start=True, stop=True)
                pt = pt_p.tile([128, 1024], BF, tag="pt", name="pt")
                nc.scalar.activation(pt[:], sp[:], EXP, 0.0, 0.125)
                pts.append(pt)
            while bgq:
                bgq.pop(0)()
            return pts

        prev = None    # (pts, pvA, pvB, kv) of pair-it i-1
        for i in range(16):
            qc, fc = divmod(i, 4)
            pts = emit_pair_scores(i, prev, bg_by_it.get(i))
            pvA = pv_p.tile([128, 512], F32, tag="pv", name=f"pvA{i}")
            pvB = pv_p.tile([128, 512], F32, tag="pv", name=f"pvB{i}")
            prev = (pts, pvA, pvB, fc // 2)
        # flush: PV(15), norm, muls, remaining outproj
        for kc in range(16):
            if kc % 3 == 0 and oqueue:
                weave_outproj()
            weave_pv(prev, kc)
        rcbA = emit_recip(prev[1])
        rcbB = emit_recip(prev[2])
        rdbA = emit_bcast(rcbA)
        emit_mul_A(15, prev[1], rdbA)
        rdbB = emit_bcast(rcbB)
        emit_mul_B(15, prev[2], rdbB)
        while oqueue:
            weave_outproj()

    nc.compile()
    return nc


def _tables():
    inv = 1.0 / (10000.0 ** (np.arange(0, 64, 2, dtype=np.float64) / 64))
    t = np.arange(S, dtype=np.float64)
    emb = np.concatenate([np.outer(t, inv)] * 2, -1)          # [S,64]
    cos_t = np.cos(emb).T.astype(np.float32)                  # [64,S]
    sin_t = np.sin(emb).T.astype(np.float32)
    ssin = np.concatenate([-sin_t[:32], sin_t[32:]], 0)
    cos_tile = np.ascontiguousarray(np.vstack([cos_t, cos_t])).astype(BF16)
    nsin_tile = np.ascontiguousarray(np.vstack([ssin, ssin])).astype(BF16)
    return cos_tile, nsin_tile


def kernel(x, Wq, Wk, Wv, Wo):
    x = np.asarray(x, np.float32)
    Wq, Wk, Wv, Wo = (np.asarray(w, np.float32) for w in (Wq, Wk, Wv, Wo))
    if "nc" not in _CACHE:
        _CACHE["nc"] = _build()
    nc = _CACHE["nc"]
    cos_tile, nsin_tile = _tables()
    eye = np.eye(128, dtype=BF16)
    xts = [np.ascontiguousarray(x[b].T).astype(BF16) for b in range(B)]

    def pack_ecols(wt):
        # [E, C] -> [128, EC*C]: chunk ec, col c at [:, C*ec + c]; each SBUF
        # row is then one contiguous 4KB-ish DMA row
        ec, c = wt.shape[0] // 128, wt.shape[1]
        return np.ascontiguousarray(
            wt.reshape(ec, 128, c).transpose(1, 0, 2).reshape(128, ec * c))

    in_maps = []
    for core in range(N_CORES):
        b, g = divmod(core, 4)
        fsl = slice(FH * g, FH * (g + 1))
        dsl = slice(128 * g, 128 * (g + 1))
        wqt = Wq[fsl].T  # [E, 512]
        # per-fc packed blocks side by side: [:, E*fc + 128*ec + c]
        wq_pack = np.concatenate(
            [pack_ecols(wqt[:, 128 * fc:128 * (fc + 1)]) for fc in range(4)],
            axis=1)
        in_maps.append({
            "xt": xts[b],
            "wqt": wq_pack.astype(BF16),
            "wkt": pack_ecols(Wk[dsl].T).astype(BF16),
            "wvt": pack_ecols(Wv[dsl].T).astype(BF16),
            "wot": np.ascontiguousarray(Wo[:, fsl].T).astype(BF16),
            "cost": cos_tile,
            "nsint": nsin_tile,
            "eye": eye,
        })
    res = run_bass_kernel_spmd(nc, in_maps, core_ids=list(range(N_CORES)),
                               **_CACHE.get("run_kwargs", {}))
    _CACHE["last_result"] = res
    _CACHE["dbg"] = [r.get("dbg") for r in res.results]
    out = np.empty((B, S, E), np.float32)
    for b in range(B):
        out[b] = sum(res.results[4 * b + g]["out"].astype(np.float32)
                     for g in range(4))
    return out
